# revision 6
# baseline (speedup 1.0000x reference)
"""Trainium2 Bass kernel for nn_AttentionLayer (sparse_attention).

Computation (per reference):
    xf = x.reshape(B, C, S);  S = W*H = 4096
    q = xf @ Wq.T + bq            [B, C, 16]
    k = xf @ Wk.T + bk            [B, C, 16]
    kq[b] = q[b] @ k[b].T         [B, C, C]
    A = softmax(kq, axis=0)       (over the batch axis -- Softmax2d)
    out[b] = A[b].T @ xf[b]       [B, C, S]

Sharding: data-parallel over batch, 2 batches per core (8 cores).  The
axis-0 softmax couples cores only through the denominator sum_b exp(kq),
exchanged via a single bf16 AllReduce.

v2 design notes (vs. the f32r v1):
  * The host ships TWO fp16 copies of x: natural [C, S] (rhs of the
    final matmul) and pre-transposed [S, C] (q/k contraction needs s on
    partitions).  This removes all 256 PE transposes and their PSUM
    evacuations; fp16 matmuls run 1 cycle/row on HW where f32r runs 2.
  * All GEMMs in fp16 (PSUM accumulates fp32): q/k quantization error
    ~5e-4 keeps the exp/softmax well inside the 2e-2 gate.
  * Output is written fp16 and upcast on the host (halves out-DMA).
  * Collective warmup AllReduce has no input dependencies (memset) so
    its trigger absorbs the cross-core launch/barrier skew early.
  * AllReduce output lives in addr_space="Shared" (fast path).
"""

import os
import numpy as np

import concourse.mybir as mybir
import concourse.tile as tile
from concourse import bacc
from concourse.bass_utils import run_bass_kernel_spmd

B, C, S, D = 16, 512, 4096, 16
N_CORES = 8
B_LOC = B // N_CORES          # 2 batches per core
CC = C // 128                 # 4 c-chunks
SC = S // 128                 # 32 s-chunks
F32 = mybir.dt.float32
F32R = mybir.dt.float32r
F16 = mybir.dt.float16
BF16 = mybir.dt.bfloat16

_CACHE = {}


def _build():
    nc = bacc.Bacc("TRN2", target_bir_lowering=False, debug=False,
                   num_devices=N_CORES)
    xT_d = nc.dram_tensor("xT", [B_LOC, S, C], F16, kind="ExternalInput")
    xn_d = nc.dram_tensor("xn", [B_LOC, C, S], F16, kind="ExternalInput")
    w_d = nc.dram_tensor("wr", [128, SC * 2 * D], F16, kind="ExternalInput")
    b_d = nc.dram_tensor("bqk", [2 * D, 1], F32, kind="ExternalInput")
    out_d = nc.dram_tensor("out", [B_LOC, C, S], F16, kind="ExternalOutput")
    rg = [list(range(N_CORES))]

    cc_in = nc.dram_tensor("cc_in", [128, CC * C], BF16, kind="Internal")
    cc_out = nc.dram_tensor("cc_out", [128, CC * C], BF16, kind="Internal",
                            addr_space="Shared")
    warm_in = nc.dram_tensor("warm_in", [128, C], BF16, kind="Internal")
    warm_out = nc.dram_tensor("warm_out", [128, C], BF16, kind="Internal",
                              addr_space="Shared")

    with tile.TileContext(nc) as tc:
        with (
            tc.tile_pool(name="persist", bufs=1) as persist,
            tc.tile_pool(name="outsb", bufs=8) as outp,
        ):
            # ---- collective warmup: zero data dependencies ----
            warm_sb = persist.tile([128, C], BF16, tag="warm", name="warm")
            nc.vector.memset(warm_sb, 1.0)
            nc.sync.dma_start(out=warm_in.ap(), in_=warm_sb)
            nc.gpsimd.collective_compute(
                "AllReduce", mybir.AluOpType.add, replica_groups=rg,
                ins=[warm_in.ap()], outs=[warm_out.ap()])

            # ---- constants ----
            wqk = persist.tile([128, SC, 2 * D], F16, tag="wqk", name="wqk")
            nc.sync.dma_start(
                out=wqk, in_=w_d.ap().rearrange("p (n d) -> p n d", n=SC))
            bqk = persist.tile([2 * D, 1], F32, tag="bqk", name="bqk")
            nc.sync.dma_start(out=bqk, in_=b_d.ap())

            # ---- x DMAs: xT first (gates q/k -> exp -> AllReduce) ----
            xT_sb = [[persist.tile([128, C], F16, tag=f"xT{b}_{sc}",
                                   name=f"xT{b}_{sc}") for sc in range(SC)]
                     for b in range(B_LOC)]
            for b in range(B_LOC):
                for sc in range(SC):
                    nc.sync.dma_start(
                        out=xT_sb[b][sc],
                        in_=xT_d.ap()[b, sc * 128:(sc + 1) * 128, :])
            xn_sb = [[persist.tile([128, S], F16, tag=f"xn{b}_{cc}",
                                   name=f"xn{b}_{cc}") for cc in range(CC)]
                     for b in range(B_LOC)]
            for b in range(B_LOC):
                for cc in range(CC):
                    nc.sync.dma_start(
                        out=xn_sb[b][cc],
                        in_=xn_d.ap()[b, cc * 128:(cc + 1) * 128, :])

            qkb_sb = [persist.tile([2 * D, C], F16, tag=f"qkb{b}",
                                   name=f"qkb{b}") for b in range(B_LOC)]
            k_sb = [persist.tile([D, C], F16, tag=f"k{b}", name=f"k{b}")
                    for b in range(B_LOC)]
            E_sb = [persist.tile([128, CC * C], F32, tag=f"E{b}",
                                 name=f"E{b}") for b in range(B_LOC)]
            A_sb = [persist.tile([128, CC * C], F16, tag=f"A{b}",
                                 name=f"A{b}") for b in range(B_LOC)]
            Sl_sb = persist.tile([128, CC * C], BF16, tag="Sl", name="Sl")
            Sg_sb = persist.tile([128, CC * C], BF16, tag="Sg", name="Sg")
            Sf_sb = persist.tile([128, CC * C], F32, tag="Sf", name="Sf")
            R_sb = persist.tile([128, CC * C], F32, tag="R", name="R")

            # ---- per batch: q/k -> kq -> exp; pair-sum + bounce ----
            with (
                tc.tile_pool(name="ps_qk", bufs=2, space="PSUM") as ps_qk,
                tc.tile_pool(name="ps_kq", bufs=2, space="PSUM") as ps_kq,
            ):
                for b in range(B_LOC):
                    qk_ps = ps_qk.tile([2 * D, C], F32)
                    for sc in range(SC):
                        nc.tensor.matmul(
                            qk_ps, lhsT=wqk[:, sc, :], rhs=xT_sb[b][sc],
                            start=(sc == 0), stop=(sc == SC - 1))
                    nc.vector.tensor_scalar_add(qkb_sb[b], qk_ps, bqk)
                    nc.sync.dma_start(out=k_sb[b], in_=qkb_sb[b][D:2 * D, :])

                    for cc in range(CC):
                        kq_ps = ps_kq.tile([128, C], F32)
                        nc.tensor.matmul(
                            kq_ps,
                            lhsT=qkb_sb[b][0:D, cc * 128:(cc + 1) * 128],
                            rhs=k_sb[b], start=True, stop=True)
                        sl = slice(cc * C, (cc + 1) * C)
                        nc.scalar.activation(
                            out=E_sb[b][:, sl], in_=kq_ps,
                            func=mybir.ActivationFunctionType.Exp)
                        if b == B_LOC - 1:
                            nc.vector.tensor_add(Sl_sb[:, sl],
                                                 E_sb[0][:, sl],
                                                 E_sb[1][:, sl])
                            nc.sync.dma_start(out=cc_in.ap()[:, sl],
                                              in_=Sl_sb[:, sl])

            # ---- single bf16 AllReduce of the local exp-sums ----
            nc.gpsimd.collective_compute(
                "AllReduce", mybir.AluOpType.add, replica_groups=rg,
                ins=[cc_in.ap()], outs=[cc_out.ap()])
            # ---- denominator, chunked readback + normalize ----
            for cc in range(CC):
                sl = slice(cc * C, (cc + 1) * C)
                nc.sync.dma_start(out=Sg_sb[:, sl], in_=cc_out.ap()[:, sl])
                nc.vector.tensor_copy(Sf_sb[:, sl], Sg_sb[:, sl])
                nc.vector.reciprocal_approx_fast(R_sb[:, sl], Sf_sb[:, sl])
                nc.vector.tensor_mul(A_sb[0][:, sl], E_sb[0][:, sl],
                                     R_sb[:, sl])
                nc.gpsimd.tensor_mul(A_sb[1][:, sl], E_sb[1][:, sl],
                                     R_sb[:, sl])

            # ---- out[b] = A[b].T @ x[b] ----
            with tc.tile_pool(name="ps_out", bufs=8, space="PSUM") as ps_out:
                for b in range(B_LOC):
                    for oc in range(CC):
                        for sg in range(2):
                            outps = [ps_out.tile([128, 512], F32,
                                                 tag="outps",
                                                 name=f"outps{j}")
                                     for j in range(4)]
                            for ic in range(CC):
                                for j in range(4):
                                    nc.tensor.matmul(
                                        outps[j],
                                        lhsT=A_sb[b][:,
                                                     ic * C + oc * 128:
                                                     ic * C + oc * 128 + 128],
                                        rhs=xn_sb[b][ic][:,
                                                         (sg * 4 + j) * 512:
                                                         (sg * 4 + j + 1) * 512],
                                        start=(ic == 0), stop=(ic == CC - 1))
                            for j in range(4):
                                sc2 = sg * 4 + j
                                o_sb = outp.tile([128, 512], F16)
                                if j % 2 == 0:
                                    nc.vector.tensor_copy(o_sb, outps[j])
                                else:
                                    nc.scalar.copy(o_sb, outps[j])
                                nc.sync.dma_start(
                                    out=out_d.ap()[b,
                                                   oc * 128:(oc + 1) * 128,
                                                   sc2 * 512:(sc2 + 1) * 512],
                                    in_=o_sb)
    nc.compile()
    return nc


def kernel(x, Wq, bq, Wk, bk):
    b_, c_, w_, h_ = x.shape
    xf16 = np.ascontiguousarray(
        x.reshape(b_, c_, w_ * h_), dtype=np.float16)           # [B, C, S]
    xT16 = np.ascontiguousarray(xf16.transpose(0, 2, 1))        # [B, S, C]
    wqkT = np.concatenate([Wq, Wk], axis=0).T.astype(np.float16)  # [S, 32]
    # [S, 2D] -> [128, SC*2D] so the weight DMA is contiguous per partition
    w_r = np.ascontiguousarray(
        wqkT.reshape(SC, 128, 2 * D).transpose(1, 0, 2).reshape(
            128, SC * 2 * D))
    bqk = np.concatenate([bq, bk]).astype(np.float32).reshape(2 * D, 1)

    if "nc" not in _CACHE:
        _CACHE["nc"] = _build()
    nc = _CACHE["nc"]

    in_maps = [
        {"xT": np.ascontiguousarray(xT16[B_LOC * j: B_LOC * (j + 1)]),
         "xn": np.ascontiguousarray(xf16[B_LOC * j: B_LOC * (j + 1)]),
         "wr": w_r, "bqk": bqk}
        for j in range(N_CORES)
    ]
    trace = bool(int(os.environ.get("BASSKERNEL_TRACE", "0")))
    res = run_bass_kernel_spmd(nc, in_maps, core_ids=list(range(N_CORES)),
                               trace=trace)
    _CACHE["last_result"] = res
    out = np.concatenate([r["out"] for r in res.results], axis=0)
    return out.astype(np.float32).reshape(b_, c_, w_, h_)


# revision 10
# speedup vs baseline: 1.1101x; 1.1101x over previous
"""Trainium2 Bass kernel for nn_AttentionLayer (sparse_attention).

Computation (per reference):
    xf = x.reshape(B, C, S);  S = W*H = 4096
    q = xf @ Wq.T + bq            [B, C, 16]
    k = xf @ Wk.T + bk            [B, C, 16]
    kq[b] = q[b] @ k[b].T         [B, C, C]
    A = softmax(kq, axis=0)       (over the batch axis -- Softmax2d)
    out[b] = A[b].T @ xf[b]       [B, C, S]

Sharding: data-parallel over batch, 2 batches per core (8 cores).  The
axis-0 softmax couples cores only through the denominator sum_b exp(kq),
exchanged via a single bf16 AllReduce.

v2 design notes (vs. the f32r v1):
  * The host ships TWO fp16 copies of x: natural [C, S] (rhs of the
    final matmul) and pre-transposed [S, C] (q/k contraction needs s on
    partitions).  This removes all 256 PE transposes and their PSUM
    evacuations; fp16 matmuls run 1 cycle/row on HW where f32r runs 2.
  * All GEMMs in fp16 (PSUM accumulates fp32): q/k quantization error
    ~5e-4 keeps the exp/softmax well inside the 2e-2 gate.
  * Output is written fp16 and upcast on the host (halves out-DMA).
  * Collective warmup AllReduce has no input dependencies (memset) so
    its trigger absorbs the cross-core launch/barrier skew early.
  * AllReduce output lives in addr_space="Shared" (fast path).
"""

import os
import numpy as np

import concourse.mybir as mybir
import concourse.tile as tile
from concourse import bacc
from concourse.bass_utils import run_bass_kernel_spmd

B, C, S, D = 16, 512, 4096, 16
N_CORES = 8
B_LOC = B // N_CORES          # 2 batches per core
CC = C // 128                 # 4 c-chunks
SC = S // 128                 # 32 s-chunks
F32 = mybir.dt.float32
F32R = mybir.dt.float32r
F16 = mybir.dt.float16
BF16 = mybir.dt.bfloat16

_CACHE = {}


def _build():
    nc = bacc.Bacc("TRN2", target_bir_lowering=False, debug=False,
                   num_devices=N_CORES)
    xT_d = nc.dram_tensor("xT", [B_LOC, S, C], F16, kind="ExternalInput")
    xn_d = nc.dram_tensor("xn", [B_LOC, C, S], F16, kind="ExternalInput")
    w_d = nc.dram_tensor("wr", [128, SC * 2 * D], F16, kind="ExternalInput")
    b_d = nc.dram_tensor("bqk", [D, 2], F32, kind="ExternalInput")
    out_d = nc.dram_tensor("out", [B_LOC, C, S], F16, kind="ExternalOutput")
    rg = [list(range(N_CORES))]

    cc_in = nc.dram_tensor("cc_in", [128, CC * C], BF16, kind="Internal")
    cc_out = nc.dram_tensor("cc_out", [128, CC * C], BF16, kind="Internal",
                            addr_space="Shared")

    with tile.TileContext(nc) as tc:
        with (
            tc.tile_pool(name="persist", bufs=1) as persist,
            tc.tile_pool(name="outsb", bufs=8) as outp,
        ):
            # ---- constants ----
            wqk = persist.tile([128, SC, 2 * D], F16, tag="wqk", name="wqk")
            nc.sync.dma_start(
                out=wqk, in_=w_d.ap().rearrange("p (n d) -> p n d", n=SC))
            bqk = persist.tile([D, 2], F32, tag="bqk", name="bqk")
            nc.sync.dma_start(out=bqk, in_=b_d.ap())

            # ---- x DMAs: xT first (gates q/k -> exp -> AllReduce) ----
            xT_sb = [[persist.tile([128, C], F16, tag=f"xT{b}_{sc}",
                                   name=f"xT{b}_{sc}") for sc in range(SC)]
                     for b in range(B_LOC)]
            for b in range(B_LOC):
                for sc in range(SC):
                    nc.sync.dma_start(
                        out=xT_sb[b][sc],
                        in_=xT_d.ap()[b, sc * 128:(sc + 1) * 128, :])
            xn_sb = [[persist.tile([128, S], F16, tag=f"xn{b}_{cc}",
                                   name=f"xn{b}_{cc}") for cc in range(CC)]
                     for b in range(B_LOC)]
            for b in range(B_LOC):
                for cc in range(CC):
                    nc.sync.dma_start(
                        out=xn_sb[b][cc],
                        in_=xn_d.ap()[b, cc * 128:(cc + 1) * 128, :])

            q_sb = [persist.tile([D, C], F16, tag=f"q{b}", name=f"q{b}")
                    for b in range(B_LOC)]
            k_sb = [persist.tile([D, C], F16, tag=f"k{b}", name=f"k{b}")
                    for b in range(B_LOC)]
            E_sb = [persist.tile([128, CC * C], F32, tag=f"E{b}",
                                 name=f"E{b}") for b in range(B_LOC)]
            A_sb = [persist.tile([128, CC * C], F16, tag=f"A{b}",
                                 name=f"A{b}") for b in range(B_LOC)]
            Sl_sb = persist.tile([128, CC * C], BF16, tag="Sl", name="Sl")
            Sf_sb = persist.tile([128, CC * C], F32, tag="Sf", name="Sf")
            R_sb = persist.tile([128, CC * C], F32, tag="R", name="R")

            # ---- per batch: q/k -> kq -> exp; pair-sum + bounce ----
            with (
                tc.tile_pool(name="ps_qk", bufs=2, space="PSUM") as ps_qk,
                tc.tile_pool(name="ps_kq", bufs=2, space="PSUM") as ps_kq,
            ):
                for b in range(B_LOC):
                    q_ps = ps_qk.tile([D, C], F32)
                    k_ps = ps_qk.tile([D, C], F32)
                    for sc in range(SC):
                        nc.tensor.matmul(
                            q_ps, lhsT=wqk[:, sc, 0:D], rhs=xT_sb[b][sc],
                            start=(sc == 0), stop=(sc == SC - 1))
                        nc.tensor.matmul(
                            k_ps, lhsT=wqk[:, sc, D:2 * D], rhs=xT_sb[b][sc],
                            start=(sc == 0), stop=(sc == SC - 1))
                    nc.vector.tensor_scalar_add(q_sb[b], q_ps,
                                                bqk[:, 0:1])
                    nc.vector.tensor_scalar_add(k_sb[b], k_ps,
                                                bqk[:, 1:2])

                    for cc in range(CC):
                        kq_ps = ps_kq.tile([128, C], F32)
                        nc.tensor.matmul(
                            kq_ps,
                            lhsT=q_sb[b][:, cc * 128:(cc + 1) * 128],
                            rhs=k_sb[b], start=True, stop=True)
                        sl = slice(cc * C, (cc + 1) * C)
                        nc.scalar.activation(
                            out=E_sb[b][:, sl], in_=kq_ps,
                            func=mybir.ActivationFunctionType.Exp)
                        if b == B_LOC - 1:
                            nc.vector.tensor_add(Sl_sb[:, sl],
                                                 E_sb[0][:, sl],
                                                 E_sb[1][:, sl])
                            nc.gpsimd.dma_start(out=cc_in.ap()[:, sl],
                                                in_=Sl_sb[:, sl])

            # ---- single bf16 AllReduce of the local exp-sums ----
            nc.gpsimd.collective_compute(
                "AllReduce", mybir.AluOpType.add, replica_groups=rg,
                ins=[cc_in.ap()], outs=[cc_out.ap()])
            # ---- denominator, chunked readback + normalize ----
            for cc in range(CC):
                sl = slice(cc * C, (cc + 1) * C)
                nc.gpsimd.dma_start(out=Sf_sb[:, sl], in_=cc_out.ap()[:, sl])
                nc.vector.reciprocal_approx_fast(R_sb[:, sl], Sf_sb[:, sl])
                nc.vector.tensor_mul(A_sb[0][:, sl], E_sb[0][:, sl],
                                     R_sb[:, sl])
                nc.gpsimd.tensor_mul(A_sb[1][:, sl], E_sb[1][:, sl],
                                     R_sb[:, sl])

            # ---- out[b] = A[b].T @ x[b] ----
            with tc.tile_pool(name="ps_out", bufs=8, space="PSUM") as ps_out:
                for b in range(B_LOC):
                    for oc in range(CC):
                        for sg in range(2):
                            outps = [ps_out.tile([128, 512], F32,
                                                 tag="outps",
                                                 name=f"outps{j}")
                                     for j in range(4)]
                            for ic in range(CC):
                                for j in range(4):
                                    nc.tensor.matmul(
                                        outps[j],
                                        lhsT=A_sb[b][:,
                                                     ic * C + oc * 128:
                                                     ic * C + oc * 128 + 128],
                                        rhs=xn_sb[b][ic][:,
                                                         (sg * 4 + j) * 512:
                                                         (sg * 4 + j + 1) * 512],
                                        start=(ic == 0), stop=(ic == CC - 1))
                            for j in range(4):
                                sc2 = sg * 4 + j
                                o_sb = outp.tile([128, 512], F16)
                                if j % 2 == 0:
                                    nc.vector.tensor_copy(o_sb, outps[j])
                                else:
                                    nc.scalar.copy(o_sb, outps[j])
                                nc.sync.dma_start(
                                    out=out_d.ap()[b,
                                                   oc * 128:(oc + 1) * 128,
                                                   sc2 * 512:(sc2 + 1) * 512],
                                    in_=o_sb)
    nc.compile()
    return nc


def kernel(x, Wq, bq, Wk, bk):
    b_, c_, w_, h_ = x.shape
    xf16 = np.ascontiguousarray(
        x.reshape(b_, c_, w_ * h_), dtype=np.float16)           # [B, C, S]
    xT16 = np.ascontiguousarray(xf16.transpose(0, 2, 1))        # [B, S, C]
    wqkT = np.concatenate([Wq, Wk], axis=0).T.astype(np.float16)  # [S, 32]
    # [S, 2D] -> [128, SC*2D] so the weight DMA is contiguous per partition
    w_r = np.ascontiguousarray(
        wqkT.reshape(SC, 128, 2 * D).transpose(1, 0, 2).reshape(
            128, SC * 2 * D))
    bqk = np.stack([bq, bk], axis=1).astype(np.float32)  # [D, 2]

    if "nc" not in _CACHE:
        _CACHE["nc"] = _build()
    nc = _CACHE["nc"]

    in_maps = [
        {"xT": np.ascontiguousarray(xT16[B_LOC * j: B_LOC * (j + 1)]),
         "xn": np.ascontiguousarray(xf16[B_LOC * j: B_LOC * (j + 1)]),
         "wr": w_r, "bqk": bqk}
        for j in range(N_CORES)
    ]
    trace = bool(int(os.environ.get("BASSKERNEL_TRACE", "0")))
    res = run_bass_kernel_spmd(nc, in_maps, core_ids=list(range(N_CORES)),
                               trace=trace)
    _CACHE["last_result"] = res
    out = np.concatenate([r["out"] for r in res.results], axis=0)
    return out.astype(np.float32).reshape(b_, c_, w_, h_)
